# revision 31
# baseline (speedup 1.0000x reference)
"""Trainium2 Bass kernel for nn_CA_5523327942720 (cross-attention, B=16).

Per sample (512ch, 64x64): energy = X1 @ X2^T (contract N=4096), softmax over
rows, attention_value = attention^T @ X1, out = concat(x1, attention_value).

Sharding: data-parallel over batch, 2 samples per NeuronCore x 8 cores.

Numerics strategy (fp32 matmul on PE is 4 cyc/row; bf16/f32r are 1):
  - mm1 (energy): bf16 hi/lo split, 3 passes (hh + hl + lh) -> ~1e-3 abs err
    on logits (vs 0.6 for plain bf16), at 3/4 the PE cost of fp32.
  - softmax in fp32 on ACT/DVE (exp with per-row -max bias, accumulated sum).
  - mm2 (attention_value): single f32r pass (E8M11, ~1e-4 rel err).
  - mm1 needs transposed operands (contraction dim n is DRAM-contiguous):
    on-chip PE transpose-mode on the bf16 hi/lo tiles (1 cyc/row).

Schedule: phase A(s) streams 32 k-tiles (load, split, transpose one tile
ahead of the mm1 matmuls); B(s) = softmax; C(s) = mm2 + output writes.
Emission order A0 B0 A1 C0 B1 C1 keeps the PE busy across phase boundaries.
"""

import numpy as np

import concourse.mybir as mybir
import concourse.tile as tile
from concourse import bacc
from concourse.bass_utils import run_bass_kernel_spmd
from concourse.masks import make_identity

f32 = mybir.dt.float32
f32r = mybir.dt.float32r
bf16 = mybir.dt.bfloat16

P = 128          # partitions
S = 2            # samples per core
C = 512          # C1 == C2
N = 4096         # H*W
KO = N // P      # 32 k-tiles for mm1 (contraction n)
CO = C // P      # 4 chunks of c1/c2
NO = N // 512    # 8 free-dim tiles for mm2
NCORES = 8

_cached_nc = None


def build_kernel():
    nc = bacc.Bacc(trn_type="TRN2")

    x1 = nc.dram_tensor("x1", [S, C, N], f32, kind="ExternalInput")
    x2 = nc.dram_tensor("x2", [S, C, N], f32, kind="ExternalInput")
    out = nc.dram_tensor("out", [S, 2 * C, N], f32, kind="ExternalOutput")
    att = nc.dram_tensor("att", [S, C, C], f32, kind="ExternalOutput")

    with tile.TileContext(nc) as tc:
        with (
            tc.tile_pool(name="singles", bufs=1) as singles,
            tc.tile_pool(name="resid", bufs=2) as resid,
            tc.tile_pool(name="chunks", bufs=5) as chunks,
            tc.tile_pool(name="chunksb", bufs=3) as chunksb,
            tc.tile_pool(name="tsb", bufs=2) as tsb,
            tc.tile_pool(name="attf", bufs=1) as attf,
            tc.tile_pool(name="attr", bufs=2) as attr,
            tc.tile_pool(name="stats", bufs=8) as stats,
            tc.tile_pool(name="avp", bufs=6) as avp,
            tc.tile_pool(name="ps_energy", bufs=4, space="PSUM") as ps_energy,
            tc.tile_pool(name="ps_tp", bufs=1, space="PSUM") as ps_tp,
            tc.tile_pool(name="ps_av", bufs=2, space="PSUM") as ps_av,
        ):
            ident = singles.tile([P, P], bf16)
            make_identity(nc, ident)

            # per-sample state carried between phases
            state = [dict() for _ in range(S)]

            def phase_a(s):
                st = state[s]
                x1_t = x1[s].rearrange("(co p) n -> p co n", p=P)
                x2_t = x2[s].rearrange("(co p) n -> p co n", p=P)
                out_x1 = out[s, 0:C, :].rearrange("(co p) n -> p co n", p=P)

                x1r = resid.tile([P, CO, N], f32r, tag="x1r", name="x1r")
                energy = [
                    ps_energy.tile([P, 512], f32, tag="energy", name=f"energy{m}")
                    for m in range(CO)
                ]
                st["x1r"] = x1r
                st["energy"] = energy

                def stage_front(k):
                    ks = slice(k * P, (k + 1) * P)
                    x1c = chunks.tile([P, CO, P], f32, tag="x1c", name="x1c")
                    x2c = chunks.tile([P, CO, P], f32, tag="x2c", name="x2c")
                    x1hc = chunksb.tile([P, CO, P], bf16, tag="x1hc", name="x1hc")
                    x1lc = chunksb.tile([P, CO, P], bf16, tag="x1lc", name="x1lc")
                    x2h = chunksb.tile([P, CO, P], bf16, tag="x2h", name="x2h")
                    x2l = chunksb.tile([P, CO, P], bf16, tag="x2l", name="x2l")
                    nc.sync.dma_start(x1c, x1_t[:, :, ks])
                    nc.sync.dma_start(x2c, x2_t[:, :, ks])
                    nc.scalar.copy(x1hc, x1c)
                    nc.vector.tensor_tensor(
                        x1lc, x1c, x1hc, mybir.AluOpType.subtract
                    )
                    nc.scalar.copy(x2h, x2c)
                    nc.vector.tensor_tensor(
                        x2l, x2c, x2h, mybir.AluOpType.subtract
                    )
                    # passthrough: out[:, 0:512, :] = x1 (SWDGE: stores never
                    # head-of-line block the SP load ring)
                    nc.gpsimd.dma_start(out_x1[:, :, ks], x1c)
                    # resident f32r copy of x1 for mm2 (DVE rounds to f32r)
                    nc.vector.tensor_copy(x1r[:, :, ks], x1c)

                    # transpose h/l of both mats: PE transpose-mode into PSUM
                    tp1 = ps_tp.tile([P, 2, 512], bf16, tag="tp1", name="tp1")
                    tp2 = ps_tp.tile([P, 2, 512], bf16, tag="tp2", name="tp2")
                    for co in range(CO):
                        cs = slice(co * P, (co + 1) * P)
                        nc.tensor.transpose(tp1[:, 0, cs], x1hc[:, co, :], ident)
                        nc.tensor.transpose(tp1[:, 1, cs], x1lc[:, co, :], ident)
                        nc.tensor.transpose(tp2[:, 0, cs], x2h[:, co, :], ident)
                        nc.tensor.transpose(tp2[:, 1, cs], x2l[:, co, :], ident)
                    # evacuate PSUM -> SBUF (x1 pair on DVE, x2 pair on ACT)
                    x1t = tsb.tile([P, 2, 512], bf16, tag="x1t", name="x1t")
                    x2t = tsb.tile([P, 2, 512], bf16, tag="x2t", name="x2t")
                    nc.vector.tensor_copy(x1t, tp1)
                    nc.scalar.copy(x2t, tp2)
                    return x1t, x2t

                def stage_mm(k, x1t, x2t):
                    for m in range(CO):
                        ms = slice(m * P, (m + 1) * P)
                        first = k == 0
                        last = k == KO - 1
                        nc.tensor.matmul(
                            energy[m], x1t[:, 0, ms], x2t[:, 0, :],
                            start=first, stop=False,
                        )
                        nc.tensor.matmul(
                            energy[m], x1t[:, 0, ms], x2t[:, 1, :],
                            start=False, stop=False,
                        )
                        nc.tensor.matmul(
                            energy[m], x1t[:, 1, ms], x2t[:, 0, :],
                            start=False, stop=last,
                        )

                pending = None
                for k in range(KO):
                    t = stage_front(k)
                    if pending is not None:
                        stage_mm(k - 1, *pending)
                    pending = t
                stage_mm(KO - 1, *pending)

            def phase_b(s):
                st = state[s]
                energy = st["energy"]
                att_f = attf.tile([P, CO, C], f32, tag="att_f", name="att_f")
                att_r = attr.tile([P, CO, C], f32r, tag="att_r", name="att_r")
                rowsum = stats.tile([P, CO], f32, tag="rowsum", name="rowsum")
                rinv = stats.tile([P, CO], f32, tag="rinv", name="rinv")
                for m in range(CO):
                    nmax = stats.tile([P, 1], f32, tag="nmax", name="nmax")
                    nc.vector.tensor_reduce(
                        nmax, energy[m], axis=mybir.AxisListType.X,
                        op=mybir.AluOpType.max, negate=True,
                    )
                    nc.scalar.activation(
                        out=att_f[:, m, :], in_=energy[m],
                        func=mybir.ActivationFunctionType.Exp,
                        bias=nmax, scale=1.0,
                        accum_out=rowsum[:, m : m + 1],
                    )
                    nc.vector.reciprocal(rinv[:, m : m + 1], rowsum[:, m : m + 1])
                    nc.vector.tensor_scalar_mul(
                        att_f[:, m, :], att_f[:, m, :], rinv[:, m : m + 1]
                    )
                    nc.vector.tensor_copy(att_r[:, m, :], att_f[:, m, :])
                nc.gpsimd.dma_start(
                    att[s].rearrange("(mo p) j -> p mo j", p=P), att_f
                )
                st["att_r"] = att_r

            def phase_c(s):
                st = state[s]
                x1r = st["x1r"]
                att_r = st["att_r"]
                out_av = out[s, C : 2 * C, :].rearrange("(jo p) n -> p jo n", p=P)
                tile_idx = 0
                for j in range(CO):
                    js = slice(j * P, (j + 1) * P)
                    for no in range(NO):
                        ns = slice(no * 512, (no + 1) * 512)
                        av = ps_av.tile([P, 512], f32, tag="av", name="av")
                        for ic in range(CO):
                            nc.tensor.matmul(
                                av, att_r[:, ic, js], x1r[:, ic, ns],
                                start=(ic == 0), stop=(ic == CO - 1),
                            )
                        av_sb = avp.tile([P, 512], f32, tag="av_sb", name="av_sb")
                        # alternate evac engine so neither ACT nor DVE gates
                        if tile_idx % 2 == 0:
                            nc.scalar.copy(av_sb, av)
                        else:
                            nc.vector.tensor_copy(av_sb, av)
                        nc.sync.dma_start(out_av[:, j, ns], av_sb)
                        tile_idx += 1

            phase_a(0)
            phase_b(0)
            phase_a(1)
            phase_c(0)
            phase_b(1)
            phase_c(1)

    nc.compile()
    return nc


def _get_nc():
    global _cached_nc
    if _cached_nc is None:
        _cached_nc = build_kernel()
    return _cached_nc


def run_sharded(x1, x2, trace=False):
    """x1, x2: (16, 512, 64, 64) fp32. Returns ((out, att), BassKernelResults)."""
    x1 = np.ascontiguousarray(np.asarray(x1, dtype=np.float32)).reshape(
        NCORES, S, C, N
    )
    x2 = np.ascontiguousarray(np.asarray(x2, dtype=np.float32)).reshape(
        NCORES, S, C, N
    )
    nc = _get_nc()
    in_maps = [{"x1": x1[c], "x2": x2[c]} for c in range(NCORES)]
    res = run_bass_kernel_spmd(
        nc, in_maps, core_ids=list(range(NCORES)), trace=trace
    )
    out = np.stack([r["out"] for r in res.results]).reshape(16, 2 * C, 64, 64)
    att = np.stack([r["att"] for r in res.results]).reshape(16, C, C)
    return (out, att), res


def kernel(x1, x2):
    (out, att), _ = run_sharded(x1, x2)
    return out, att


# revision 32
# speedup vs baseline: 1.0021x; 1.0021x over previous
"""Trainium2 Bass kernel for nn_CA_5523327942720 (cross-attention, B=16).

Per sample (512ch, 64x64): energy = X1 @ X2^T (contract N=4096), softmax over
rows, attention_value = attention^T @ X1, out = concat(x1, attention_value).

Sharding: data-parallel over batch, 2 samples per NeuronCore x 8 cores.

Numerics strategy (fp32 matmul on PE is 4 cyc/row; bf16/f32r are 1):
  - mm1 (energy): bf16 hi/lo split, 3 passes (hh + hl + lh) -> ~1e-3 abs err
    on logits (vs 0.6 for plain bf16), at 3/4 the PE cost of fp32.
  - softmax in fp32 on ACT/DVE (exp with per-row -max bias, accumulated sum).
  - mm2 (attention_value): single f32r pass (E8M11, ~1e-4 rel err).
  - mm1 needs transposed operands (contraction dim n is DRAM-contiguous):
    on-chip PE transpose-mode on the bf16 hi/lo tiles (1 cyc/row).

Schedule: phase A(s) streams 32 k-tiles (load, split, transpose one tile
ahead of the mm1 matmuls); B(s) = softmax; C(s) = mm2 + output writes.
Emission order A0 B0 A1 C0 B1 C1 keeps the PE busy across phase boundaries.
"""

import numpy as np

import concourse.mybir as mybir
import concourse.tile as tile
from concourse import bacc
from concourse.bass_utils import run_bass_kernel_spmd
from concourse.masks import make_identity

f32 = mybir.dt.float32
f32r = mybir.dt.float32r
bf16 = mybir.dt.bfloat16

P = 128          # partitions
S = 2            # samples per core
C = 512          # C1 == C2
N = 4096         # H*W
KO = N // P      # 32 k-tiles for mm1 (contraction n)
CO = C // P      # 4 chunks of c1/c2
NO = N // 512    # 8 free-dim tiles for mm2
NCORES = 8

_cached_nc = None


def build_kernel():
    nc = bacc.Bacc(trn_type="TRN2")

    x1 = nc.dram_tensor("x1", [S, C, N], f32, kind="ExternalInput")
    x2 = nc.dram_tensor("x2", [S, C, N], f32, kind="ExternalInput")
    out = nc.dram_tensor("out", [S, 2 * C, N], f32, kind="ExternalOutput")
    att = nc.dram_tensor("att", [S, C, C], f32, kind="ExternalOutput")

    with tile.TileContext(nc) as tc:
        with (
            tc.tile_pool(name="singles", bufs=1) as singles,
            tc.tile_pool(name="resid", bufs=2) as resid,
            tc.tile_pool(name="chunks", bufs=4) as chunks,
            tc.tile_pool(name="tsb", bufs=2) as tsb,
            tc.tile_pool(name="attf", bufs=1) as attf,
            tc.tile_pool(name="attr", bufs=2) as attr,
            tc.tile_pool(name="stats", bufs=8) as stats,
            tc.tile_pool(name="avp", bufs=6) as avp,
            tc.tile_pool(name="ps_energy", bufs=4, space="PSUM") as ps_energy,
            tc.tile_pool(name="ps_tp", bufs=1, space="PSUM") as ps_tp,
            tc.tile_pool(name="ps_av", bufs=2, space="PSUM") as ps_av,
        ):
            ident = singles.tile([P, P], bf16)
            make_identity(nc, ident)

            # per-sample state carried between phases
            state = [dict() for _ in range(S)]

            def phase_a(s):
                st = state[s]
                x1_t = x1[s].rearrange("(co p) n -> p co n", p=P)
                x2_t = x2[s].rearrange("(co p) n -> p co n", p=P)
                out_x1 = out[s, 0:C, :].rearrange("(co p) n -> p co n", p=P)

                x1r = resid.tile([P, CO, N], f32r, tag="x1r", name="x1r")
                energy = [
                    ps_energy.tile([P, 512], f32, tag="energy", name=f"energy{m}")
                    for m in range(CO)
                ]
                st["x1r"] = x1r
                st["energy"] = energy

                def stage_front(k):
                    ks = slice(k * P, (k + 1) * P)
                    x1c = chunks.tile([P, CO, P], f32, tag="x1c", name="x1c")
                    x2c = chunks.tile([P, CO, P], f32, tag="x2c", name="x2c")
                    x1hc = chunks.tile([P, CO, P], bf16, tag="x1hc", name="x1hc")
                    x1lc = chunks.tile([P, CO, P], bf16, tag="x1lc", name="x1lc")
                    x2h = chunks.tile([P, CO, P], bf16, tag="x2h", name="x2h")
                    x2l = chunks.tile([P, CO, P], bf16, tag="x2l", name="x2l")
                    nc.sync.dma_start(x1c, x1_t[:, :, ks])
                    nc.sync.dma_start(x2c, x2_t[:, :, ks])
                    nc.scalar.copy(x1hc, x1c)
                    nc.vector.tensor_tensor(
                        x1lc, x1c, x1hc, mybir.AluOpType.subtract
                    )
                    nc.scalar.copy(x2h, x2c)
                    nc.vector.tensor_tensor(
                        x2l, x2c, x2h, mybir.AluOpType.subtract
                    )
                    # passthrough: out[:, 0:512, :] = x1 (SWDGE: stores never
                    # head-of-line block the SP load ring)
                    nc.gpsimd.dma_start(out_x1[:, :, ks], x1c)
                    # resident f32r copy of x1 for mm2 (DVE rounds to f32r)
                    nc.vector.tensor_copy(x1r[:, :, ks], x1c)

                    # transpose h/l of both mats: PE transpose-mode into PSUM
                    tp1 = ps_tp.tile([P, 2, 512], bf16, tag="tp1", name="tp1")
                    tp2 = ps_tp.tile([P, 2, 512], bf16, tag="tp2", name="tp2")
                    for co in range(CO):
                        cs = slice(co * P, (co + 1) * P)
                        nc.tensor.transpose(tp1[:, 0, cs], x1hc[:, co, :], ident)
                        nc.tensor.transpose(tp1[:, 1, cs], x1lc[:, co, :], ident)
                        nc.tensor.transpose(tp2[:, 0, cs], x2h[:, co, :], ident)
                        nc.tensor.transpose(tp2[:, 1, cs], x2l[:, co, :], ident)
                    # evacuate PSUM -> SBUF (x1 pair on DVE, x2 pair on ACT)
                    x1t = tsb.tile([P, 2, 512], bf16, tag="x1t", name="x1t")
                    x2t = tsb.tile([P, 2, 512], bf16, tag="x2t", name="x2t")
                    nc.vector.tensor_copy(x1t, tp1)
                    nc.scalar.copy(x2t, tp2)
                    return x1t, x2t

                def stage_mm(k, x1t, x2t):
                    for m in range(CO):
                        ms = slice(m * P, (m + 1) * P)
                        first = k == 0
                        last = k == KO - 1
                        nc.tensor.matmul(
                            energy[m], x1t[:, 0, ms], x2t[:, 0, :],
                            start=first, stop=False,
                        )
                        nc.tensor.matmul(
                            energy[m], x1t[:, 0, ms], x2t[:, 1, :],
                            start=False, stop=False,
                        )
                        nc.tensor.matmul(
                            energy[m], x1t[:, 1, ms], x2t[:, 0, :],
                            start=False, stop=last,
                        )

                pending = None
                for k in range(KO):
                    t = stage_front(k)
                    if pending is not None:
                        stage_mm(k - 1, *pending)
                    pending = t
                stage_mm(KO - 1, *pending)

            def phase_b(s):
                st = state[s]
                energy = st["energy"]
                att_f = attf.tile([P, CO, C], f32, tag="att_f", name="att_f")
                att_r = attr.tile([P, CO, C], f32r, tag="att_r", name="att_r")
                rowsum = stats.tile([P, CO], f32, tag="rowsum", name="rowsum")
                rinv = stats.tile([P, CO], f32, tag="rinv", name="rinv")
                for m in range(CO):
                    nmax = stats.tile([P, 1], f32, tag="nmax", name="nmax")
                    nc.vector.tensor_reduce(
                        nmax, energy[m], axis=mybir.AxisListType.X,
                        op=mybir.AluOpType.max, negate=True,
                    )
                    nc.scalar.activation(
                        out=att_f[:, m, :], in_=energy[m],
                        func=mybir.ActivationFunctionType.Exp,
                        bias=nmax, scale=1.0,
                        accum_out=rowsum[:, m : m + 1],
                    )
                    nc.vector.reciprocal(rinv[:, m : m + 1], rowsum[:, m : m + 1])
                    nc.vector.tensor_scalar_mul(
                        att_f[:, m, :], att_f[:, m, :], rinv[:, m : m + 1]
                    )
                    nc.vector.tensor_copy(att_r[:, m, :], att_f[:, m, :])
                nc.gpsimd.dma_start(
                    att[s].rearrange("(mo p) j -> p mo j", p=P), att_f
                )
                st["att_r"] = att_r

            def phase_c(s):
                st = state[s]
                x1r = st["x1r"]
                att_r = st["att_r"]
                out_av = out[s, C : 2 * C, :].rearrange("(jo p) n -> p jo n", p=P)
                tile_idx = 0
                for j in range(CO):
                    js = slice(j * P, (j + 1) * P)
                    for no in range(NO):
                        ns = slice(no * 512, (no + 1) * 512)
                        av = ps_av.tile([P, 512], f32, tag="av", name="av")
                        for ic in range(CO):
                            nc.tensor.matmul(
                                av, att_r[:, ic, js], x1r[:, ic, ns],
                                start=(ic == 0), stop=(ic == CO - 1),
                            )
                        av_sb = avp.tile([P, 512], f32, tag="av_sb", name="av_sb")
                        # alternate evac engine so neither ACT nor DVE gates
                        if tile_idx % 2 == 0:
                            nc.scalar.copy(av_sb, av)
                        else:
                            nc.vector.tensor_copy(av_sb, av)
                        nc.sync.dma_start(out_av[:, j, ns], av_sb)
                        tile_idx += 1

            phase_a(0)
            phase_b(0)
            phase_a(1)
            phase_c(0)
            phase_b(1)
            phase_c(1)

    nc.compile()
    return nc


def _get_nc():
    global _cached_nc
    if _cached_nc is None:
        _cached_nc = build_kernel()
    return _cached_nc


def run_sharded(x1, x2, trace=False):
    """x1, x2: (16, 512, 64, 64) fp32. Returns ((out, att), BassKernelResults)."""
    x1 = np.ascontiguousarray(np.asarray(x1, dtype=np.float32)).reshape(
        NCORES, S, C, N
    )
    x2 = np.ascontiguousarray(np.asarray(x2, dtype=np.float32)).reshape(
        NCORES, S, C, N
    )
    nc = _get_nc()
    in_maps = [{"x1": x1[c], "x2": x2[c]} for c in range(NCORES)]
    res = run_bass_kernel_spmd(
        nc, in_maps, core_ids=list(range(NCORES)), trace=trace
    )
    out = np.stack([r["out"] for r in res.results]).reshape(16, 2 * C, 64, 64)
    att = np.stack([r["att"] for r in res.results]).reshape(16, C, C)
    return (out, att), res


def kernel(x1, x2):
    (out, att), _ = run_sharded(x1, x2)
    return out, att
